# revision 36
# baseline (speedup 1.0000x reference)
"""Trainium2 Bass kernel for nn_Decoder: fused single-step LSTM decoder.

Reference computation (per token t of batch b, state never advances):
    gates = x[b,t] @ W_ih.T + (h0[b] @ W_hh.T + b_ih + b_hh)     # [2048]
    i, f, g, o = sigmoid/sigmoid/tanh/sigmoid of gate quarters
    c = f * c0[b] + i * g
    h = o * tanh(c)
    out[b,t] = h @ fc_w.T + fc_b                                 # [513]

Sharding: data-parallel, batch 64 -> 8 batches per core on 8 NeuronCores.
Per-core layout strategy ("feature-major"):
  - x is cast to bf16 on the host and zero-padded to 640 columns; the
    kernel never loads it natively. Each 512-token tile is brought in as
    5 DMA xbar-transposes (DRAM [512 tok, 128 d] -> SBUF [128 d, 512]),
    so the transpose costs zero PE/DVE time. The 5th window [512:640)
    holds feature 512 at row 0 (rest zero padding).
  - gates are computed transposed in bf16: gatesT[g-chunk, tok] with
    host-cast bf16 W_ihT as the stationary operand; the K=513
    contraction is 4 full K=128 chunks + one K=1 matmul for feature
    512. The per-batch bias const (h0 @ W_hh.T + b_ih + b_hh,
    precomputed fp32 on host) enters for free as the ScalarE activation
    per-partition bias AP.
  - LSTM cell math runs feature-major in fp32 so c0 is a per-partition
    scalar; h (cast to bf16) lands directly in the [h, tok] layout the
    fc matmul needs as lhsT.
  - fc runs in bf16 on h: out[tok, 513] token-major in two N-halves
    (258+258 into one 2-bank PSUM tile, cols 513..515 zero-pad), fc_b
    folded in via a K=1 ones-row matmul. One ScalarE copy PSUM->SBUF
    per subtile, DMA out per 128-token subtile.
  - All DMAs (including the xbar transposes) issue on the single SP
    queue: mixing transpose-mode and copy-mode DMAs across queues hits
    a TRN2 xbar hardware bug (silent corruption or device hang).
"""

from contextlib import ExitStack

import ml_dtypes
import numpy as np

import concourse.bass as bass
import concourse.tile as tile
from concourse import bacc, mybir
from concourse.bass_utils import run_bass_kernel_spmd

FP32 = mybir.dt.float32
FP32R = mybir.dt.float32r
BF16 = mybir.dt.bfloat16
AFT = mybir.ActivationFunctionType

N_CORES = 8
B, T, D = 64, 1024, 513
H = 512
G4 = 4 * H  # 2048
B_LOC = B // N_CORES  # 8 batches per core
TOK = B_LOC * T  # 8192 tokens per core
TT = 512  # tokens per tile (stays within one batch: T % TT == 0)
NT = TOK // TT  # 16 tiles
NM = TT // 128  # 4 token-subtiles of 128
DX = 640  # host-padded x width: 5 transpose windows of 128 (cols 513+ zero)
DPAD = 516  # fc output padded width (cols 513..515 zero garbage)
NSPLIT = [258, 258]  # fc N split halves (each fits one PSUM bank)


def build_nc(reps=1):
    nc = bacc.Bacc("TRN2", target_bir_lowering=False, debug=False, num_devices=N_CORES)
    x = nc.dram_tensor("x", [TOK, DX], BF16, kind="ExternalInput").ap()
    wih_t = nc.dram_tensor("wih_t", [H, G4], BF16, kind="ExternalInput").ap()
    wih_row = nc.dram_tensor("wih_row", [1, G4], BF16, kind="ExternalInput").ap()
    fc_rhs = nc.dram_tensor("fc_rhs", [H, DPAD], BF16, kind="ExternalInput").ap()
    fc_row = nc.dram_tensor("fc_row", [1, DPAD], BF16, kind="ExternalInput").ap()
    bct = nc.dram_tensor("bconst", [128, 16 * B_LOC], FP32, kind="ExternalInput").ap()
    c0t = nc.dram_tensor("c0t", [128, 4 * B_LOC], FP32, kind="ExternalInput").ap()
    out = nc.dram_tensor("out", [TOK, D], FP32, kind="ExternalOutput").ap()

    with tile.TileContext(nc) as tc, ExitStack() as ctx:
        const = ctx.enter_context(tc.tile_pool(name="const", bufs=1))
        xtp = ctx.enter_context(tc.tile_pool(name="xt", bufs=15))
        sigp = ctx.enter_context(tc.tile_pool(name="sig", bufs=10))
        tmpp = ctx.enter_context(tc.tile_pool(name="tmp", bufs=2))
        hp = ctx.enter_context(tc.tile_pool(name="h", bufs=8))
        outp = ctx.enter_context(tc.tile_pool(name="osb", bufs=4))
        # PSUM: 8 banks: gates 4 + fc 2x2-bank
        gpp = ctx.enter_context(tc.tile_pool(name="gp", bufs=4, space="PSUM"))
        fmp = ctx.enter_context(tc.tile_pool(name="fm", bufs=2, space="PSUM"))

        def load_weights():
            wsb = []
            for k in range(4):
                w = const.tile([128, G4], BF16, tag=f"w{k}", name=f"w{k}")
                nc.sync.dma_start(w[:], wih_t[k * 128 : (k + 1) * 128, :])
                wsb.append(w)
            w4 = const.tile([1, G4], BF16, tag="w4")
            nc.sync.dma_start(w4[:], wih_row)
            wsb.append(w4)
            return wsb

        def load_fc_chunk(k):
            w = const.tile([128, DPAD], BF16, tag=f"f{k}", name=f"f{k}")
            nc.sync.dma_start(w[:], fc_rhs[k * 128 : (k + 1) * 128, :])
            return w

        gate_funcs = [AFT.Sigmoid, AFT.Sigmoid, AFT.Tanh, AFT.Sigmoid]

        def emit_transposes(tt):
            """Bring in tile tt as 5 transposed bf16 chunks via DMA xbar."""
            ts = tt * TT
            xt = []
            for k in range(5):
                t = xtp.tile([128, TT], BF16, tag="xt")
                nc.sync.dma_start_transpose(
                    t[:], x[ts : ts + TT, k * 128 : (k + 1) * 128]
                )
                xt.append(t)
            return xt

        # ---- startup, DMAs ordered by first use on the single queue:
        # xt0 (first transposes) -> W_ih (first gates) -> bct/c0 (first
        # activations) -> fc weights -> xt1 ----
        xt_queue = [emit_transposes(0)]
        wsb = load_weights()

        bct_sb = const.tile([128, 16 * B_LOC], FP32, tag="bct")
        c0_sb = const.tile([128, 4 * B_LOC], FP32, tag="c0")
        nc.sync.dma_start(bct_sb[:], bct)
        nc.sync.dma_start(c0_sb[:], c0t)

        fcsb = [load_fc_chunk(k) for k in range(4)]
        f4 = const.tile([1, DPAD], BF16, tag="f4")
        nc.sync.dma_start(f4[:], fc_row)
        fcsb.append(f4)

        ones = const.tile([1, 128], BF16, tag="ones")
        nc.vector.memset(ones[:], 1.0)

        xt_queue.append(emit_transposes(1))

        # ---- main loop over 16 token tiles of 512 ----
        # (optionally repeated `reps` times inside one NEFF for timing)
        rep_ctx = tc.For_i(0, reps, 1) if reps > 1 else None
        if rep_ctx is not None:
            rep_ctx.__enter__()
        for tt in range(NT):
            b = tt // (T // TT)
            ts = tt * TT
            xt = xt_queue.pop(0)

            # prefetch + transpose two tiles ahead while this tile computes
            # (reps>1 wraps around so each For_i iteration is steady-state)
            if reps > 1:
                xt_queue.append(emit_transposes((tt + 2) % NT))
            elif tt + 2 < NT:
                xt_queue.append(emit_transposes(tt + 2))

            # gates + LSTM cell, per h-chunk k. kd-outer: all four gate
            # chunks accumulate together across 4 PSUM banks so consecutive
            # matmuls share the same rhs tile and weights are consumed in
            # arrival order at startup.
            hn = []
            for k in range(4):
                pss = [
                    gpp.tile([128, TT], FP32, tag="gp", name=f"gp_{tt}_{k}_{gi}")
                    for gi in range(4)
                ]
                for kd in range(5):
                    for gi in range(4):
                        c = gi * 4 + k  # g-chunk index in [0, 16)
                        if kd < 4:
                            nc.tensor.matmul(
                                pss[gi][:],
                                wsb[kd][:, c * 128 : (c + 1) * 128],
                                xt[kd][:],
                                start=(kd == 0),
                                stop=False,
                            )
                        else:
                            nc.tensor.matmul(
                                pss[gi][:],
                                wsb[4][:, c * 128 : (c + 1) * 128],
                                xt[4][0:1, :],
                                start=False,
                                stop=True,
                            )
                gs = []
                for gi in range(4):
                    c = gi * 4 + k
                    s = sigp.tile([128, TT], FP32, tag="sig")
                    nc.scalar.activation(
                        s[:],
                        pss[gi][:],
                        gate_funcs[gi],
                        bias=bct_sb[:, c * B_LOC + b : c * B_LOC + b + 1],
                    )
                    gs.append(s)
                i_s, f_s, g_s, o_s = gs
                t1 = tmpp.tile([128, TT], FP32, tag="t1")
                nc.vector.tensor_mul(t1[:], i_s[:], g_s[:])
                t2 = tmpp.tile([128, TT], FP32, tag="t2")
                nc.vector.tensor_scalar_mul(
                    t2[:], f_s[:], c0_sb[:, k * B_LOC + b : k * B_LOC + b + 1]
                )
                cc = tmpp.tile([128, TT], FP32, tag="cc")
                nc.vector.tensor_add(cc[:], t1[:], t2[:])
                th = tmpp.tile([128, TT], FP32, tag="th")
                nc.scalar.activation(th[:], cc[:], AFT.Tanh)
                h = hp.tile([128, TT], BF16, tag="h")
                nc.vector.tensor_mul(h[:], o_s[:], th[:])
                hn.append(h)

            # fc: out[tok, 513] per 128-token subtile, N split 256 + 258
            osb = outp.tile([128, NM, DPAD], FP32, tag="osb")
            for m in range(NM):
                msl = slice(m * 128, (m + 1) * 128)
                lhs5 = [
                    hn[0][:, msl],
                    hn[1][:, msl],
                    hn[2][:, msl],
                    hn[3][:, msl],
                    ones[:],
                ]
                pf = fmp.tile([128, 1024], FP32, tag="fm")
                for kd in range(5):
                    nc.tensor.matmul(
                        pf[:, 0 : NSPLIT[0]],
                        lhs5[kd],
                        fcsb[kd][:, 0 : NSPLIT[0]],
                        start=(kd == 0),
                        stop=(kd == 4),
                    )
                for kd in range(5):
                    nc.tensor.matmul(
                        pf[:, 512 : 512 + NSPLIT[1]],
                        lhs5[kd],
                        fcsb[kd][:, NSPLIT[0] : DPAD],
                        start=(kd == 0),
                        stop=(kd == 4),
                    )
                nc.scalar.copy(
                    osb[:, m, 0:DPAD].rearrange("p (a b) -> p a b", a=2),
                    pf[:].rearrange("p (a b) -> p a b", a=2)[:, :, 0 : NSPLIT[0]],
                )
                nc.sync.dma_start(
                    out[ts + m * 128 : ts + (m + 1) * 128, :], osb[:, m, 0:D]
                )
        if rep_ctx is not None:
            rep_ctx.__exit__(None, None, None)

    nc.compile()
    return nc


_NC_CACHE = []


def get_nc():
    if not _NC_CACHE:
        _NC_CACHE.append(build_nc())
    return _NC_CACHE[0]


def make_in_maps(decoder_inputs, h0, c0, W_ih, W_hh, b_ih, b_hh, fc_w, fc_b):
    di = np.asarray(decoder_inputs, dtype=np.float32)
    h0 = np.asarray(h0, dtype=np.float32)[0]  # [64, 512]
    c0 = np.asarray(c0, dtype=np.float32)[0]
    W_ih = np.asarray(W_ih, dtype=np.float32)
    W_hh = np.asarray(W_hh, dtype=np.float32)
    b_ih = np.asarray(b_ih, dtype=np.float32)
    b_hh = np.asarray(b_hh, dtype=np.float32)
    fc_w = np.asarray(fc_w, dtype=np.float32)
    fc_b = np.asarray(fc_b, dtype=np.float32)

    bc = h0 @ W_hh.T + b_ih + b_hh  # [64, 2048]
    wih_tp = np.ascontiguousarray(W_ih.T[0:512]).astype(ml_dtypes.bfloat16)
    wih_row_a = np.ascontiguousarray(W_ih.T[512:513]).astype(ml_dtypes.bfloat16)
    fc_pad = np.zeros((H, DPAD), dtype=ml_dtypes.bfloat16)
    fc_pad[:, 0:D] = fc_w.T.astype(ml_dtypes.bfloat16)
    fc_row_a = np.zeros((1, DPAD), dtype=ml_dtypes.bfloat16)
    fc_row_a[0, 0:D] = fc_b.astype(ml_dtypes.bfloat16)

    # x: bf16 cast + zero-pad to 640 cols (transpose windows of 128)
    x_pad = np.zeros((B * T, DX), dtype=ml_dtypes.bfloat16)
    x_pad[:, 0:D] = di.reshape(B * T, D).astype(ml_dtypes.bfloat16)

    in_maps = []
    for core in range(N_CORES):
        bs = core * B_LOC
        xc = x_pad[bs * T : (bs + B_LOC) * T]
        # bct[p, c*8+b] = bc[bs+b, c*128+p]
        bct = np.ascontiguousarray(
            bc[bs : bs + B_LOC]
            .reshape(B_LOC, 16, 128)
            .transpose(2, 1, 0)
            .reshape(128, -1)
        )
        c0c = np.ascontiguousarray(
            c0[bs : bs + B_LOC]
            .reshape(B_LOC, 4, 128)
            .transpose(2, 1, 0)
            .reshape(128, -1)
        )
        in_maps.append(
            {
                "x": xc,
                "wih_t": wih_tp,
                "wih_row": wih_row_a,
                "fc_rhs": fc_pad,
                "fc_row": fc_row_a,
                "bconst": bct,
                "c0t": c0c,
            }
        )
    return in_maps


def kernel(**inputs):
    in_maps = make_in_maps(**inputs)
    nc = get_nc()
    res = run_bass_kernel_spmd(nc, in_maps, core_ids=list(range(N_CORES)))
    out = np.concatenate([res.results[c]["out"] for c in range(N_CORES)], axis=0)
    return out.reshape(B, T, D)


# revision 37
# speedup vs baseline: 1.1479x; 1.1479x over previous
"""Trainium2 Bass kernel for nn_Decoder: fused single-step LSTM decoder.

Reference computation (per token t of batch b, state never advances):
    gates = x[b,t] @ W_ih.T + (h0[b] @ W_hh.T + b_ih + b_hh)     # [2048]
    i, f, g, o = sigmoid/sigmoid/tanh/sigmoid of gate quarters
    c = f * c0[b] + i * g
    h = o * tanh(c)
    out[b,t] = h @ fc_w.T + fc_b                                 # [513]

Sharding: data-parallel, batch 64 -> 8 batches per core on 8 NeuronCores.
Per-core layout strategy ("feature-major"):
  - x is cast to bf16 on the host and zero-padded to 640 columns; the
    kernel never loads it natively. Each 512-token tile is brought in as
    5 DMA xbar-transposes (DRAM [512 tok, 128 d] -> SBUF [128 d, 512]),
    so the transpose costs zero PE/DVE time. The 5th window [512:640)
    holds feature 512 at row 0 (rest zero padding).
  - gates are computed transposed in bf16: gatesT[g-chunk, tok] with
    host-cast bf16 W_ihT as the stationary operand; the K=513
    contraction is 4 full K=128 chunks + one K=1 matmul for feature
    512. The per-batch bias const (h0 @ W_hh.T + b_ih + b_hh,
    precomputed fp32 on host) enters for free as the ScalarE activation
    per-partition bias AP.
  - LSTM cell math runs feature-major in fp32 so c0 is a per-partition
    scalar; h (cast to bf16) lands directly in the [h, tok] layout the
    fc matmul needs as lhsT.
  - fc runs in bf16 on h: out[tok, 513] token-major in two N-halves
    (258+258 into one 2-bank PSUM tile, cols 513..515 zero-pad), fc_b
    folded in via a K=1 ones-row matmul. One ScalarE copy PSUM->SBUF
    per subtile, DMA out per 128-token subtile.
  - All DMAs (including the xbar transposes) issue on the single SP
    queue: mixing transpose-mode and copy-mode DMAs across queues hits
    a TRN2 xbar hardware bug (silent corruption or device hang).
"""

from contextlib import ExitStack

import ml_dtypes
import numpy as np

import concourse.bass as bass
import concourse.tile as tile
from concourse import bacc, mybir
from concourse.bass_utils import run_bass_kernel_spmd

FP32 = mybir.dt.float32
FP32R = mybir.dt.float32r
BF16 = mybir.dt.bfloat16
AFT = mybir.ActivationFunctionType

N_CORES = 8
B, T, D = 64, 1024, 513
H = 512
G4 = 4 * H  # 2048
B_LOC = B // N_CORES  # 8 batches per core
TOK = B_LOC * T  # 8192 tokens per core
TT = 512  # tokens per tile (stays within one batch: T % TT == 0)
NT = TOK // TT  # 16 tiles
NM = TT // 128  # 4 token-subtiles of 128
DX = 640  # host-padded x width: 5 transpose windows of 128 (cols 513+ zero)
DPAD = 516  # fc output padded width (cols 513..515 zero garbage)
NSPLIT = [258, 258]  # fc N split halves (each fits one PSUM bank)


def build_nc(reps=1):
    nc = bacc.Bacc("TRN2", target_bir_lowering=False, debug=False, num_devices=N_CORES)
    x = nc.dram_tensor("x", [TOK, DX], BF16, kind="ExternalInput").ap()
    wih_t = nc.dram_tensor("wih_t", [H, G4], BF16, kind="ExternalInput").ap()
    wih_row = nc.dram_tensor("wih_row", [1, G4], BF16, kind="ExternalInput").ap()
    fc_rhs = nc.dram_tensor("fc_rhs", [H, DPAD], BF16, kind="ExternalInput").ap()
    fc_row = nc.dram_tensor("fc_row", [1, DPAD], BF16, kind="ExternalInput").ap()
    bct = nc.dram_tensor("bconst", [128, 16 * B_LOC], FP32, kind="ExternalInput").ap()
    c0t = nc.dram_tensor("c0t", [128, 4 * B_LOC], FP32, kind="ExternalInput").ap()
    out = nc.dram_tensor("out", [TOK, D], FP32, kind="ExternalOutput").ap()

    with tile.TileContext(nc) as tc, ExitStack() as ctx:
        const = ctx.enter_context(tc.tile_pool(name="const", bufs=1))
        xtp = ctx.enter_context(tc.tile_pool(name="xt", bufs=15))
        sigp = ctx.enter_context(tc.tile_pool(name="sig", bufs=10))
        tmpp = ctx.enter_context(tc.tile_pool(name="tmp", bufs=2))
        hp = ctx.enter_context(tc.tile_pool(name="h", bufs=8))
        outp = ctx.enter_context(tc.tile_pool(name="osb", bufs=4))
        # PSUM: 8 banks: gates 4 + fc 2x2-bank
        gpp = ctx.enter_context(tc.tile_pool(name="gp", bufs=4, space="PSUM"))
        fmp = ctx.enter_context(tc.tile_pool(name="fm", bufs=2, space="PSUM"))

        def load_weights():
            wsb = []
            for k in range(4):
                w = const.tile([128, G4], BF16, tag=f"w{k}", name=f"w{k}")
                nc.sync.dma_start(w[:], wih_t[k * 128 : (k + 1) * 128, :])
                wsb.append(w)
            w4 = const.tile([1, G4], BF16, tag="w4")
            nc.sync.dma_start(w4[:], wih_row)
            wsb.append(w4)
            return wsb

        def load_fc_chunk(k):
            w = const.tile([128, DPAD], BF16, tag=f"f{k}", name=f"f{k}")
            nc.sync.dma_start(w[:], fc_rhs[k * 128 : (k + 1) * 128, :])
            return w

        gate_funcs = [AFT.Sigmoid, AFT.Sigmoid, AFT.Tanh, AFT.Sigmoid]

        def emit_transposes(tt):
            """Bring in tile tt as 5 transposed bf16 chunks via DMA xbar."""
            ts = tt * TT
            xt = []
            for k in range(5):
                t = xtp.tile([128, TT], BF16, tag="xt")
                nc.sync.dma_start_transpose(
                    t[:], x[ts : ts + TT, k * 128 : (k + 1) * 128]
                )
                xt.append(t)
            return xt

        # ---- startup, DMAs ordered by first use on the single queue:
        # xt0 (first transposes) -> W_ih (first gates) -> bct/c0 (first
        # activations) -> fc weights -> xt1 ----
        xt_queue = [emit_transposes(0)]
        wsb = load_weights()

        bct_sb = const.tile([128, 16 * B_LOC], FP32, tag="bct")
        c0_sb = const.tile([128, 4 * B_LOC], FP32, tag="c0")
        nc.sync.dma_start(bct_sb[:], bct)
        nc.sync.dma_start(c0_sb[:], c0t)

        fcsb = [load_fc_chunk(k) for k in range(4)]
        f4 = const.tile([1, DPAD], BF16, tag="f4")
        nc.sync.dma_start(f4[:], fc_row)
        fcsb.append(f4)

        ones = const.tile([1, 128], BF16, tag="ones")
        nc.vector.memset(ones[:], 1.0)

        xt_queue.append(emit_transposes(1))

        # ---- main loop over 16 token tiles of 512 ----
        # (optionally repeated `reps` times inside one NEFF for timing)
        rep_ctx = tc.For_i(0, reps, 1) if reps > 1 else None
        if rep_ctx is not None:
            rep_ctx.__enter__()
        for tt in range(NT):
            b = tt // (T // TT)
            ts = tt * TT
            xt = xt_queue.pop(0)

            # prefetch + transpose two tiles ahead while this tile computes
            # (reps>1 wraps around so each For_i iteration is steady-state)
            if reps > 1:
                xt_queue.append(emit_transposes((tt + 2) % NT))
            elif tt + 2 < NT:
                xt_queue.append(emit_transposes(tt + 2))

            # gates + LSTM cell, per h-chunk k
            hn = []
            for k in range(4):
                gs = []
                for gi in range(4):
                    c = gi * 4 + k  # g-chunk index in [0, 16)
                    ps = gpp.tile([128, TT], FP32, tag="gp")
                    for kd in range(4):
                        nc.tensor.matmul(
                            ps[:],
                            wsb[kd][:, c * 128 : (c + 1) * 128],
                            xt[kd][:],
                            start=(kd == 0),
                            stop=False,
                        )
                    nc.tensor.matmul(
                        ps[:],
                        wsb[4][:, c * 128 : (c + 1) * 128],
                        xt[4][0:1, :],
                        start=False,
                        stop=True,
                    )
                    s = sigp.tile([128, TT], FP32, tag="sig")
                    nc.scalar.activation(
                        s[:],
                        ps[:],
                        gate_funcs[gi],
                        bias=bct_sb[:, c * B_LOC + b : c * B_LOC + b + 1],
                    )
                    gs.append(s)
                i_s, f_s, g_s, o_s = gs
                t1 = tmpp.tile([128, TT], FP32, tag="t1")
                nc.vector.tensor_mul(t1[:], i_s[:], g_s[:])
                t2 = tmpp.tile([128, TT], FP32, tag="t2")
                nc.vector.tensor_scalar_mul(
                    t2[:], f_s[:], c0_sb[:, k * B_LOC + b : k * B_LOC + b + 1]
                )
                cc = tmpp.tile([128, TT], FP32, tag="cc")
                nc.vector.tensor_add(cc[:], t1[:], t2[:])
                th = tmpp.tile([128, TT], FP32, tag="th")
                nc.scalar.activation(th[:], cc[:], AFT.Tanh)
                h = hp.tile([128, TT], BF16, tag="h")
                nc.vector.tensor_mul(h[:], o_s[:], th[:])
                hn.append(h)

            # fc: out[tok, 513] per 128-token subtile, N split 256 + 258
            osb = outp.tile([128, NM, DPAD], FP32, tag="osb")
            for m in range(NM):
                msl = slice(m * 128, (m + 1) * 128)
                lhs5 = [
                    hn[0][:, msl],
                    hn[1][:, msl],
                    hn[2][:, msl],
                    hn[3][:, msl],
                    ones[:],
                ]
                pf = fmp.tile([128, 1024], FP32, tag="fm")
                for kd in range(5):
                    nc.tensor.matmul(
                        pf[:, 0 : NSPLIT[0]],
                        lhs5[kd],
                        fcsb[kd][:, 0 : NSPLIT[0]],
                        start=(kd == 0),
                        stop=(kd == 4),
                    )
                for kd in range(5):
                    nc.tensor.matmul(
                        pf[:, 512 : 512 + NSPLIT[1]],
                        lhs5[kd],
                        fcsb[kd][:, NSPLIT[0] : DPAD],
                        start=(kd == 0),
                        stop=(kd == 4),
                    )
                nc.scalar.copy(
                    osb[:, m, 0:DPAD].rearrange("p (a b) -> p a b", a=2),
                    pf[:].rearrange("p (a b) -> p a b", a=2)[:, :, 0 : NSPLIT[0]],
                )
                nc.sync.dma_start(
                    out[ts + m * 128 : ts + (m + 1) * 128, :], osb[:, m, 0:D]
                )
        if rep_ctx is not None:
            rep_ctx.__exit__(None, None, None)

    nc.compile()
    return nc


_NC_CACHE = []


def get_nc():
    if not _NC_CACHE:
        _NC_CACHE.append(build_nc())
    return _NC_CACHE[0]


def make_in_maps(decoder_inputs, h0, c0, W_ih, W_hh, b_ih, b_hh, fc_w, fc_b):
    di = np.asarray(decoder_inputs, dtype=np.float32)
    h0 = np.asarray(h0, dtype=np.float32)[0]  # [64, 512]
    c0 = np.asarray(c0, dtype=np.float32)[0]
    W_ih = np.asarray(W_ih, dtype=np.float32)
    W_hh = np.asarray(W_hh, dtype=np.float32)
    b_ih = np.asarray(b_ih, dtype=np.float32)
    b_hh = np.asarray(b_hh, dtype=np.float32)
    fc_w = np.asarray(fc_w, dtype=np.float32)
    fc_b = np.asarray(fc_b, dtype=np.float32)

    bc = h0 @ W_hh.T + b_ih + b_hh  # [64, 2048]
    wih_tp = np.ascontiguousarray(W_ih.T[0:512]).astype(ml_dtypes.bfloat16)
    wih_row_a = np.ascontiguousarray(W_ih.T[512:513]).astype(ml_dtypes.bfloat16)
    fc_pad = np.zeros((H, DPAD), dtype=ml_dtypes.bfloat16)
    fc_pad[:, 0:D] = fc_w.T.astype(ml_dtypes.bfloat16)
    fc_row_a = np.zeros((1, DPAD), dtype=ml_dtypes.bfloat16)
    fc_row_a[0, 0:D] = fc_b.astype(ml_dtypes.bfloat16)

    # x: bf16 cast + zero-pad to 640 cols (transpose windows of 128)
    x_pad = np.zeros((B * T, DX), dtype=ml_dtypes.bfloat16)
    x_pad[:, 0:D] = di.reshape(B * T, D).astype(ml_dtypes.bfloat16)

    in_maps = []
    for core in range(N_CORES):
        bs = core * B_LOC
        xc = x_pad[bs * T : (bs + B_LOC) * T]
        # bct[p, c*8+b] = bc[bs+b, c*128+p]
        bct = np.ascontiguousarray(
            bc[bs : bs + B_LOC]
            .reshape(B_LOC, 16, 128)
            .transpose(2, 1, 0)
            .reshape(128, -1)
        )
        c0c = np.ascontiguousarray(
            c0[bs : bs + B_LOC]
            .reshape(B_LOC, 4, 128)
            .transpose(2, 1, 0)
            .reshape(128, -1)
        )
        in_maps.append(
            {
                "x": xc,
                "wih_t": wih_tp,
                "wih_row": wih_row_a,
                "fc_rhs": fc_pad,
                "fc_row": fc_row_a,
                "bconst": bct,
                "c0t": c0c,
            }
        )
    return in_maps


def kernel(**inputs):
    in_maps = make_in_maps(**inputs)
    nc = get_nc()
    res = run_bass_kernel_spmd(nc, in_maps, core_ids=list(range(N_CORES)))
    out = np.concatenate([res.results[c]["out"] for c in range(N_CORES)], axis=0)
    return out.reshape(B, T, D)
